# revision 62
# baseline (speedup 1.0000x reference)
"""Multi-head attention TRN2 kernel (nn_Attention_48859547959768).

Head-parallel tensor parallelism across 8 NeuronCores: each core computes
2 of the 16 heads end-to-end (column-parallel QKV projection, attention,
row-parallel output projection) and returns a partial [B,S,DIM] output;
the host sums the 8 partials and adds the output bias.

Per-core dataflow (all matmuls bf16):
  - X^T staged in SBUF (both batches resident, xtp bufs=2*KT). Batch 0's
    X is split across the sync and scalar DMA queues (weights wq/wk/wv
    lead the scalar queue); batch 1's X follows batch 0's on the sync
    FIFO so the head's load gets full DMA bandwidth first.
  - Q^T/K^T = W^T @ X^T computed kt-outer (each matmul waits only on its
    own X tile DMA) with K/Q interleaved per span; span0's bias-adds run
    on ScalarE (idle pre-exp) so the first score pair never waits behind
    the serialized DVE bias chain.
  - Q^T/K^T live as [128, S] tiles: partitions 0-63 = head0's 64 dims,
    64-127 = head1's. Score matmuls for the two heads run CONCURRENTLY
    via tile_position row-tiling (K=64 each), writing one [128,1024] PSUM.
  - exp on ScalarE with 1/sqrt(D) folded into the activation scale and
    the mask penalty as per-partition bias; no row-max subtraction
    (scores are O(1)). ScalarE is the pacing engine (~148us of exp per
    core): the whole attention runs as ONE continuous 128-iteration loop
    (no span/group boundaries) with PV drains, span evacuations and
    normalizations emitted lazily so no boundary burst ever stalls the
    exp stream. All other PE work interleaves as fine-grained "filler"
    units; dummy matmuls keep the PE HAM clock gate at K=8/8 when the
    fillers run dry.
  - V^T -> V chunk transposes stay on the PE for both batches: chunk
    production lives in the PE FIFO (PV can never outrun it) and the
    ~4.4us of PE time is far cheaper than the span-handover stalls the
    DMA-xbar transpose convoy caused on the sync queue.
  - ctx'^T[dv,q] = V'.T @ exp^T with a 65th V column of ones so row 64
    accumulates the softmax denominators for free.
  - per-span normalization: reciprocal denominators broadcast to the 64
    head dims via gpsimd; out-proj is plain K=128 matmuls with single-copy
    PSUM evacuation. batch1 span3 normalizes during out-proj evacuation
    (ScalarE idle after the last exp; the tail's po pairs alternate
    between the ps_po and ps_pc pools so consecutive units pipeline).
  - per-core partial outputs are written bf16 and summed on host in f32.
"""

import numpy as np

B, S, DIM = 2, 2048, 1024
H, D = 16, 64
N_CORES = 8
HPC = H // N_CORES  # heads per core = 2
DHC = HPC * D       # per-core head-dim slice = 128
KT = DIM // 128     # contraction tiles for projections = 8
NSPAN = S // 512    # q spans = 4
NCHUNK = S // 128   # 128-token chunks = 16
CPS = 4             # chunks per span

_cached = {}


def _build():
    from collections import deque

    import concourse.mybir as mybir
    from concourse import bacc
    from concourse.masks import make_identity
    from concourse.tile import TileContext

    f32 = mybir.dt.float32
    bf16 = mybir.dt.bfloat16
    dt = bf16
    Exp = mybir.ActivationFunctionType.Exp
    Mult = mybir.AluOpType.mult
    Add = mybir.AluOpType.add

    nc = bacc.Bacc("TRN2", target_bir_lowering=False)

    xt = nc.dram_tensor("xt", [B, DIM, S], dt, kind="ExternalInput").ap()
    pen = nc.dram_tensor("pen", [B, 128, NCHUNK], f32, kind="ExternalInput").ap()
    wq = nc.dram_tensor("wq", [DIM, DHC], dt, kind="ExternalInput").ap()
    wk = nc.dram_tensor("wk", [DIM, DHC], dt, kind="ExternalInput").ap()
    wv = nc.dram_tensor("wv", [DIM, DHC], dt, kind="ExternalInput").ap()
    wo = nc.dram_tensor("wo", [DHC, DIM], dt, kind="ExternalInput").ap()
    bqd = nc.dram_tensor("bq", [DHC, 1], f32, kind="ExternalInput").ap()
    bkd = nc.dram_tensor("bk", [DHC, 1], f32, kind="ExternalInput").ap()
    bvd = nc.dram_tensor("bv", [DHC, 1], f32, kind="ExternalInput").ap()
    out = nc.dram_tensor("out", [B, S, DIM], dt, kind="ExternalOutput").ap()

    with TileContext(nc) as tc:
        from contextlib import ExitStack

        with ExitStack() as ctx:
            const = ctx.enter_context(tc.tile_pool(name="const", bufs=1))
            xtp = ctx.enter_context(tc.tile_pool(name="xtp", bufs=2 * KT))
            persist = ctx.enter_context(tc.tile_pool(name="persist", bufs=1))
            ctxp = ctx.enter_context(tc.tile_pool(name="ctxp", bufs=2))
            work = ctx.enter_context(tc.tile_pool(name="work", bufs=3))
            ps_sc = ctx.enter_context(tc.tile_pool(name="ps_sc", bufs=2, space="PSUM"))
            ps_pc = ctx.enter_context(tc.tile_pool(name="ps_pc", bufs=2, space="PSUM"))
            ps_po = ctx.enter_context(tc.tile_pool(name="ps_po", bufs=2, space="PSUM"))

            # PE warmup: keep TensorE busy through the initial DMA load so
            # HAM reaches K=8/8 before the projections start. Uses a DVE-
            # memset tile: make_identity's gpsimd iota pays the ~6us Q7
            # first-call IRAM load and must NOT gate the warmup.
            wtile = const.tile([128, 128], dt)
            nc.vector.memset(wtile, 0.0)
            wps = ps_po.tile([128, 512], f32, tag="po", name="wps")
            for _ in range(28):
                nc.tensor.matmul(wps[:, 0:128], wtile, wtile,
                                 start=True, stop=True)
            ident = const.tile([128, 128], dt)
            make_identity(nc, ident)
            onef = const.tile([1, 1], f32)
            nc.vector.memset(onef, 1.0)
            ones128 = const.tile([128, 1], dt)
            nc.vector.memset(ones128, 1.0)
            # wq/wk/wv lead the scalar DMA queue (needed first by the
            # projections); X tiles for batch 0 split across sync+scalar.
            wq_sb = const.tile([128, KT, DHC], dt)
            wk_sb = const.tile([128, KT, DHC], dt)
            wv_sb = const.tile([128, KT, DHC], dt)
            nc.scalar.dma_start(out=wq_sb, in_=wq.rearrange("(kt p) m -> p kt m", p=128))
            nc.scalar.dma_start(out=wk_sb, in_=wk.rearrange("(kt p) m -> p kt m", p=128))
            nc.scalar.dma_start(out=wv_sb, in_=wv.rearrange("(kt p) m -> p kt m", p=128))

            # ---- per-batch phase functions (emitted in pipelined order) ----
            st = [dict(spans_done=set(), vpc=0) for _ in range(B)]

            def phase_load(b):
                with nc.named_scope(f"load{b}"):
                    xt_t = []
                    for kt in range(KT):
                        t = xtp.tile([128, S], dt, tag="xt", name=f"xt{kt}")
                        half = S // 2
                        for hh in range(2):
                            eng = nc.scalar if (b == 0 and kt >= 5) else nc.sync
                            eng.dma_start(
                                out=t[:, hh * half:(hh + 1) * half],
                                in_=xt[b, kt * 128:(kt + 1) * 128,
                                       hh * half:(hh + 1) * half])
                        xt_t.append(t)
                    st[b]["xt"] = xt_t
                    pen_sb = work.tile([128, NCHUNK], f32, tag="pen", name="pen")
                    nc.sync.dma_start(out=pen_sb, in_=pen[b])
                    st[b]["pen"] = pen_sb

            def load_late_consts():
                # biases + wo go on the SYNC queue after batch0's X halves
                # (before batch1's X): the scalar queue then carries ONLY
                # wq/wk/wv + batch0's kt5-7 X halves, so the scheduler has
                # nothing to reorder ahead of those halves (it used to put
                # pen/biases first, landing kt5-7 at ~29us instead of ~16).
                bq_sb = const.tile([128, 1], f32)
                bk_sb = const.tile([128, 1], f32)
                bv_sb = const.tile([128, 1], f32)
                nc.sync.dma_start(out=bk_sb, in_=bkd)
                nc.sync.dma_start(out=bq_sb, in_=bqd)
                nc.sync.dma_start(out=bv_sb, in_=bvd)
                wo_sb = const.tile([128, DIM], dt)
                nc.sync.dma_start(out=wo_sb, in_=wo)
                return bq_sb, bk_sb, bv_sb, wo_sb

            def proj_qk_ktouter(b):
                # kt-outer: each matmul waits only on X tile kt, so the PE
                # starts right after the first DMA lands. K/Q interleaved
                # per span so span0's bias-adds (and the first scores)
                # overlap the projection tail. 8 concurrent accumulators
                # live in the (otherwise idle) attention pools.
                xt_t = st[b]["xt"]
                qtp = persist.tile([128, S], dt, tag="qtp", name="qtp", bufs=2)
                ktp = persist.tile([128, S], dt, tag="ktp", name="ktp", bufs=2)
                with nc.named_scope(f"proj{b}"):
                    scA = ps_sc.tile([128, 1024], f32, tag="sc", name="scA")
                    scB = ps_sc.tile([128, 1024], f32, tag="sc", name="scB")
                    pcA = ps_pc.tile([128, 512], f32, tag="pc", name="pcA")
                    pcB = ps_pc.tile([128, 512], f32, tag="pc", name="pcB")
                    poA = ps_po.tile([128, 512], f32, tag="po", name="poA")
                    poB = ps_po.tile([128, 512], f32, tag="po", name="poB")
                    qacc = [scA[:, 0:512], scA[:, 512:1024],
                            scB[:, 0:512], scB[:, 512:1024]]
                    kacc = [pcA, pcB, poA, poB]
                    for kt in range(KT):
                        xts = xt_t[kt]
                        for sp in range(NSPAN):
                            nc.tensor.matmul(
                                kacc[sp], wk_sb[:, kt, :],
                                xts[:, sp * 512:(sp + 1) * 512],
                                start=(kt == 0), stop=(kt == KT - 1))
                            nc.tensor.matmul(
                                qacc[sp], wq_sb[:, kt, :],
                                xts[:, sp * 512:(sp + 1) * 512],
                                start=(kt == 0), stop=(kt == KT - 1))
                    for sp in range(NSPAN):
                        if sp == 0:
                            # span0's bias-adds run on ScalarE (idle before
                            # the first exp) so the first score pair never
                            # waits behind the serialized DVE bias chain
                            nc.scalar.add(out=ktp[:, 0:512], in_=kacc[0],
                                          add=bk_sb[:, 0:1])
                            nc.scalar.add(out=qtp[:, 0:512], in_=qacc[0],
                                          add=bq_sb[:, 0:1])
                        else:
                            nc.vector.tensor_scalar_add(
                                out=ktp[:, sp * 512:(sp + 1) * 512],
                                in0=kacc[sp], scalar1=bk_sb[:, 0:1])
                            nc.vector.tensor_scalar_add(
                                out=qtp[:, sp * 512:(sp + 1) * 512],
                                in0=qacc[sp], scalar1=bq_sb[:, 0:1])
                st[b]["qtp"], st[b]["ktp"] = qtp, ktp

            def gen_v_and_transp(b):
                # V projection as PE filler units; V chunk transposes go
                # through the DMA xbar (sync queue), not the PE.
                # st[b]["vpc"] counts emitted transposes: phase_attn's PV
                # matmuls are gated on it (emission order = dependency
                # order in Tile).
                xt_t = st[b]["xt"]
                vt = persist.tile([128, S], dt, tag="vt", name="vt", bufs=2)
                vp = persist.tile([128, NCHUNK, HPC, 65], dt,
                                  tag="vp", name="vp", bufs=2)
                st[b]["vp"] = vp
                nc.vector.memset(vp[:, :, :, 64:65], 1.0)
                for sp in range(NSPAN):
                    ps = ps_po.tile([128, 512], f32, tag="po", name="ps")
                    for kt in range(KT):
                        nc.tensor.matmul(
                            ps, wv_sb[:, kt, :],
                            xt_t[kt][:, sp * 512:(sp + 1) * 512],
                            start=(kt == 0), stop=(kt == KT - 1))
                        # fine-grained filler units (2 matmuls each) so one
                        # fill slot never injects a ~0.9us PE burst into an
                        # attention iteration
                        if kt % 2 == 1 and kt < KT - 1:
                            yield
                    nc.vector.tensor_scalar_add(
                        out=vt[:, sp * 512:(sp + 1) * 512],
                        in0=ps, scalar1=bv_sb[:, 0:1])
                    yield
                    for c in range(sp * CPS, (sp + 1) * CPS):
                        # PE-path transpose for both batches: chunk
                        # production stays in the PE FIFO (PV can never
                        # outrun it) and avoids the sync-queue transpose
                        # convoy that kept stalling span handovers.
                        pt = ps_po.tile([128, 512], dt, tag="po",
                                        name="pt")
                        nc.tensor.transpose(
                            pt[:, 0:128], vt[:, c * 128:(c + 1) * 128],
                            ident)
                        nc.vector.tensor_copy(out=vp[:, c, :, 0:64],
                                              in_=pt[:, 0:128])
                        st[b]["vpc"] += 1
                        yield

            def gen_load(b):
                phase_load(b)
                return
                yield

            def gen_proj_qk(b):
                # span-outer q/k projection as filler units (uses only the 2
                # po slots; runs inside another batch's attention phase).
                xt_t = st[b]["xt"]
                qtp = persist.tile([128, S], dt, tag="qtp", name="qtp", bufs=2)
                ktp = persist.tile([128, S], dt, tag="ktp", name="ktp", bufs=2)
                st[b]["qtp"], st[b]["ktp"] = qtp, ktp
                for w_sb, b_sb, dst in ((wq_sb, bq_sb, qtp),
                                        (wk_sb, bk_sb, ktp)):
                    for sp in range(NSPAN):
                        ps = ps_po.tile([128, 512], f32, tag="po", name="ps")
                        for kt in range(KT):
                            nc.tensor.matmul(
                                ps, w_sb[:, kt, :],
                                xt_t[kt][:, sp * 512:(sp + 1) * 512],
                                start=(kt == 0), stop=(kt == KT - 1))
                            if kt % 2 == 1 and kt < KT - 1:
                                yield
                        nc.vector.tensor_scalar_add(
                            out=dst[:, sp * 512:(sp + 1) * 512],
                            in0=ps, scalar1=b_sb[:, 0:1])
                        yield

            def _fill(filler):
                # Generators yield True after emitting real PE work and
                # False when blocked (nothing emitted). Returns False when
                # no real work was emitted so the caller can keep the PE
                # HAM-warm with a dummy matmul.
                if filler is None:
                    return False
                for _ in range(8):
                    try:
                        v = next(filler)
                    except StopIteration:
                        return False
                    if v is not False:
                        return True
                return False

            def emit_scores_g(b2, sp2, kt2):
                qtp2, ktp2 = st[b2]["qtp"], st[b2]["ktp"]
                sc = ps_sc.tile([128, 1024], f32, tag="sc", name="sc")
                for h in range(HPC):
                    psl = slice(h * 64, (h + 1) * 64)
                    nc.tensor.matmul(
                        sc[:, h * 512:(h + 1) * 512],
                        ktp2[psl, kt2 * 128:(kt2 + 1) * 128],
                        qtp2[psl, sp2 * 512:(sp2 + 1) * 512],
                        start=True, stop=True,
                    )
                return sc

            def finish_span(b2, sp2, pc):
                # evacuate the span's ctx + denominator rows, then normalize
                # (1/den broadcast to the 64 head dims via SBUF-SBUF tile
                # DMAs + gpsimd broadcast - all dependency-tracked, the
                # gpsimd engine is idle). batch1 span3 skips the norm: the
                # post-attention tail normalizes during out-proj evacuation
                # (the DMA round-trip chain would sit exposed after the last
                # exp with nothing to hide it).
                ctxt, ctxtn = st[b2]["ctxt"], st[b2]["ctxtn"]
                qsl = slice(sp2 * 512, (sp2 + 1) * 512)
                for h in range(HPC):
                    nc.vector.tensor_copy(
                        out=ctxt[h * 64:(h + 1) * 64, qsl],
                        in_=pc[h][0:64, :])
                    denrow = work.tile([1, 512], f32, tag="den",
                                       name="denrow", bufs=4)
                    nc.vector.tensor_copy(out=denrow, in_=pc[h][64:65, :])
                    st[b2][f"denrow{h}"] = denrow
                for h in range(HPC) if not (b2 == 1 and sp2 == 3) else ():
                    hsl = slice(h * 64, (h + 1) * 64)
                    rtmp = work.tile([128, CPS], f32, tag="rtmp",
                                     name="rtmp", bufs=4)
                    nc.sync.dma_start(
                        out=rtmp,
                        in_=st[b2][f"denrow{h}"].rearrange(
                            "o (p c) -> o p c", p=128))
                    rec4 = work.tile([128, CPS], f32, tag="rec4",
                                     name="rec4", bufs=4)
                    nc.vector.reciprocal(rec4, rtmp)
                    rrow = work.tile([1, 512], f32, tag="rrow",
                                     name="rrow", bufs=4)
                    nc.sync.dma_start(
                        out=rrow.rearrange("o (p c) -> o p c", p=128),
                        in_=rec4)
                    # broadcast to all 128 partitions and slice the head's
                    # range so the multiply's inputs share the same
                    # partitions (verifier requirement)
                    rt = work.tile([128, 512], f32, tag="rt",
                                   name="rt", bufs=4)
                    nc.gpsimd.partition_broadcast(out_ap=rt, in_ap=rrow)
                    nc.vector.tensor_mul(out=ctxtn[hsl, qsl],
                                         in0=ctxt[hsl, qsl],
                                         in1=rt[hsl, :])
                st[b2]["spans_done"].add(sp2)

            def phase_attn_all(filler=None, fill_from=0, fill2_until=0):
                # ONE continuous loop over all B*NSPAN*NCHUNK iterations: per
                # kv chunk, paired row-tiled score matmuls -> one [128,1024]
                # exp -> PV matmuls. PV drains lag the exp stream through a
                # pending deque (gated on st[b]["vpc"], the V-transpose
                # chunks emitted so far) and catch up at 3 per iteration;
                # the 16-slot e ring decouples the two streams. A span's
                # evacuation + normalization are emitted lazily when its
                # last PV drains - a few iterations INTO the next span - so
                # there is no span-boundary burst stalling the exp stream.
                # When the filler chain is exhausted, dummy matmuls keep the
                # PE HAM clock gate warm.
                for b2 in range(B):
                    st[b2]["ctxt"] = ctxp.tile([128, S], dt, tag="ctxt",
                                               name="ctxt")
                    st[b2]["ctxtn"] = ctxp.tile([128, S], dt, tag="ctxtn",
                                                name="ctxtn")
                seq = [(b2, 2 * g + si, kt)
                       for b2 in range(B) for g in range(2)
                       for si in range(2) for kt in range(NCHUNK)]
                pending = deque()  # entries: (b, sp, kt, e_tile)
                pc_of = {}
                npv = {}
                fin_state = {"it": -10}

                def emit_pv_one(cur_it):
                    eb, es, ek, pe = pending.popleft()
                    key = (eb, es)
                    if key not in pc_of:
                        pc_of[key] = [ps_pc.tile([65, 512], f32, tag="pc",
                                                 name=f"pc{i}")
                                      for i in range(HPC)]
                    pc = pc_of[key]
                    vp = st[eb]["vp"]
                    for h in range(HPC):
                        nc.tensor.matmul(
                            pc[h], vp[:, ek, h, :],
                            pe[:, h * 512:(h + 1) * 512],
                            start=(ek == 0), stop=(ek == NCHUNK - 1))
                    npv[key] = npv.get(key, 0) + 1
                    if npv[key] == NCHUNK:
                        fin_state["it"] = cur_it
                        finish_span(eb, es, pc_of.pop(key))

                def head_ready(cur_it):
                    if not pending:
                        return False
                    eb, es, ek, _ = pending[0]
                    v = st[eb]["vpc"]
                    # batch1 lags 1 chunk behind the emitted xbar transposes
                    # so a PV matmul never head-of-line-blocks the PE queue
                    # on an in-flight transpose; once all 16 transposes are
                    # emitted the lag MUST drop or the last chunk would
                    # never drain (runtime deadlock through the e-ring).
                    lag = 1 if (eb and v < NCHUNK) else 0
                    if ek >= v - lag:
                        return False
                    # a NEW span's first PV reuses the pc banks of the span
                    # evacuated two spans ago... with bufs=2 and 2 tiles per
                    # span it actually waits on the PREVIOUS span's evac -
                    # give that evac 2 iterations of runtime headroom so the
                    # waiting matmul never head-of-line-blocks the PE queue
                    if ek == 0 and cur_it < fin_state["it"] + 2:
                        return False
                    return True

                with nc.named_scope("attn"):
                    sc_next = emit_scores_g(*seq[0])
                    for it, (b2, sp2, kt) in enumerate(seq):
                        sc = sc_next
                        n = 0
                        while n < 4 and head_ready(it):
                            emit_pv_one(it)
                            n += 1
                        e = work.tile([128, 1024], dt, tag="expT",
                                      name="e", bufs=24)
                        nc.scalar.activation(e, sc, Exp, scale=0.125,
                                             bias=st[b2]["pen"][:, kt:kt + 1])
                        pending.append((b2, sp2, kt, e))
                        if it + 1 < len(seq):
                            sc_next = emit_scores_g(*seq[it + 1])
                        if it >= fill_from:
                            ok = _fill(filler)
                            # spread the catch-up double-fills over every
                            # other iteration so they never push an
                            # iteration's PE work past the exp pace
                            if ok and it < 2 * fill2_until and it % 2 == 1:
                                _fill(filler)
                            if not ok:
                                # HAM keep-warm dummy
                                wd = ps_po.tile([128, 512], f32,
                                                tag="po", name="wd")
                                nc.tensor.matmul(wd[:, 0:128], wtile,
                                                 wtile, start=True,
                                                 stop=True)
                    while pending:
                        emit_pv_one(len(seq))

            def gen_outproj(b, chunks, scalar_assist=False):
                # out[c*128+t, :] = ctxn^T[:, t].T @ Wo as plain K=128
                # matmuls over the normalized ctx; evacuation is a single
                # cast-copy per osp (on ScalarE for the post-attention tail,
                # where ScalarE is idle).
                for c in chunks:
                    # a chunk's instructions may only be EMITTED once its
                    # span's normalization has been emitted (Tile orders by
                    # emission; a read emitted before any write sees garbage)
                    while (c // CPS) not in st[b]["spans_done"]:
                        yield False
                    ctxtn = st[b]["ctxtn"]
                    csl = slice(c * 128, (c + 1) * 128)
                    ob = work.tile([128, DIM], dt, tag="ob", name="ob", bufs=4)
                    for osp in range(2):
                        jsl = slice(osp * 512, (osp + 1) * 512)
                        po = ps_po.tile([128, 512], f32, tag="po", name="po")
                        nc.tensor.matmul(
                            po, ctxtn[:, csl], wo_sb[:, jsl],
                            start=True, stop=True)
                        if scalar_assist:
                            nc.scalar.copy(out=ob[:, jsl], in_=po)
                        else:
                            nc.vector.tensor_copy(out=ob[:, jsl], in_=po)
                        yield
                    nc.sync.dma_start(out=out[b, csl, :], in_=ob)

            def gen_outproj_tail(b):
                # final span (chunks 12-15): no broadcast chain - build
                # chunk-major 1/den via K=1 PE transposes of the den rows
                # (keeps the PE busy and HAM warm right after attention) and
                # normalize during evacuation: ob = po0*r0[t] + po1*r1[t],
                # split across ScalarE (idle after the last exp) and DVE.
                # The po pairs alternate between the ps_po and ps_pc pools
                # (both idle in the tail) so unit k+1's matmuls never wait
                # on unit k's evacuation.
                ctxt = st[b]["ctxt"]
                recs = []
                for h in range(HPC):
                    dps = ps_pc.tile([128, CPS], f32, tag="pc", name="dps")
                    for j in range(CPS):
                        nc.tensor.transpose(
                            dps[:, j:j + 1],
                            st[b][f"denrow{h}"][:, j * 128:(j + 1) * 128],
                            onef)
                    rsb = work.tile([128, CPS], f32, tag="rec4", name="rsb",
                                    bufs=4)
                    nc.vector.reciprocal(rsb, dps)
                    recs.append(rsb)
                    # keep the PE HAM clock gate at K=8/8 through the
                    # flush/evac/reciprocal lull so the out-proj matmuls
                    # below run at 2.4GHz instead of re-warming from 1.2
                    wt = ps_po.tile([128, 512], f32, tag="po", name="wt")
                    for _ in range(10):
                        nc.tensor.matmul(wt[:, 0:128], wtile, wtile,
                                         start=True, stop=True)
                    yield
                unit = 0
                for c in range(3 * CPS, NCHUNK):
                    j = c - 3 * CPS
                    csl = slice(c * 128, (c + 1) * 128)
                    ob = work.tile([128, DIM], dt, tag="ob", name="ob", bufs=4)
                    for osp in range(2):
                        jsl = slice(osp * 512, (osp + 1) * 512)
                        pool, ptag = ((ps_po, "po") if unit % 2 == 0
                                      else (ps_pc, "pc"))
                        unit += 1
                        po = [pool.tile([128, 512], f32, tag=ptag,
                                        name=f"tpo{h}")
                              for h in range(HPC)]
                        for h in range(HPC):
                            psl = slice(h * 64, (h + 1) * 64)
                            nc.tensor.matmul(
                                po[h], ctxt[psl, csl], wo_sb[psl, jsl],
                                start=True, stop=True)
                        tmp = work.tile([128, 512], f32, tag="obt",
                                        name="tmp")
                        nc.scalar.mul(tmp, po[0], recs[0][:, j:j + 1])
                        nc.vector.scalar_tensor_tensor(
                            out=ob[:, jsl], in0=po[1],
                            scalar=recs[1][:, j:j + 1],
                            in1=tmp, op0=Mult, op1=Add)
                        yield
                    nc.sync.dma_start(out=out[b, csl, :], in_=ob)

            def drain(filler):
                for _ in filler:
                    pass

            def chain(*gens):
                for gg in gens:
                    yield from gg

            # ---- emission schedule ----
            # batch0's X load + QK projection run up front; batch1's X load
            # is the FIRST filler (xtp is double-capacity so it never waits
            # on buffer recycling), then batch0's V projection/transposes,
            # then batch1's q/k projection and the out-projections.
            phase_load(0)
            bq_sb, bk_sb, bv_sb, wo_sb = load_late_consts()
            proj_qk_ktouter(0)

            chainA = chain(gen_load(1), gen_v_and_transp(0),
                           gen_v_and_transp(1), gen_proj_qk(1),
                           gen_outproj(0, range(NCHUNK)),
                           gen_outproj(1, range(0, 3 * CPS)))
            phase_attn_all(filler=chainA, fill_from=0, fill2_until=26)
            drain(chainA)
            drain(gen_outproj_tail(1))

    nc.compile()
    return nc


def _get_nc():
    if "nc" not in _cached:
        _cached["nc"] = _build()
    return _cached["nc"]


def kernel(X, mask, Wq, bq, Wk, bk, Wv, bv, Wo, bo):
    import ml_dtypes
    from concourse.bass_utils import run_bass_kernel_spmd

    bf = ml_dtypes.bfloat16
    X = np.asarray(X, dtype=np.float32)
    mask = np.asarray(mask, dtype=np.float32)
    Wq, Wk, Wv, Wo = (np.asarray(a, dtype=np.float32) for a in (Wq, Wk, Wv, Wo))
    bq, bk, bv, bo = (np.asarray(a, dtype=np.float32) for a in (bq, bk, bv, bo))

    xtf = np.ascontiguousarray(X.transpose(0, 2, 1)).astype(bf)  # [B, DIM, S]
    pen_full = (-1e6 * (1.0 - mask)).astype(np.float32)      # [B, S]
    # bias tile layout: pen_sb[p, kt] = pen_full[b, kt*128 + p]
    penf = np.ascontiguousarray(
        pen_full.reshape(B, NCHUNK, 128).transpose(0, 2, 1))

    in_maps = []
    for c in range(N_CORES):
        sl = slice(c * DHC, (c + 1) * DHC)
        in_maps.append({
            "xt": xtf,
            "pen": penf,
            "wq": np.ascontiguousarray(Wq[:, sl]).astype(bf),
            "wk": np.ascontiguousarray(Wk[:, sl]).astype(bf),
            "wv": np.ascontiguousarray(Wv[:, sl]).astype(bf),
            "wo": np.ascontiguousarray(Wo[sl, :]).astype(bf),
            "bq": np.ascontiguousarray(bq[sl].reshape(DHC, 1)),
            "bk": np.ascontiguousarray(bk[sl].reshape(DHC, 1)),
            "bv": np.ascontiguousarray(bv[sl].reshape(DHC, 1)),
        })

    res = run_bass_kernel_spmd(_get_nc(), in_maps, core_ids=list(range(N_CORES)))
    _cached["last_results"] = res
    acc = res.results[0]["out"].astype(np.float32)
    for c in range(1, N_CORES):
        acc = acc + res.results[c]["out"].astype(np.float32)
    acc += bo[None, None, :]
    return acc.astype(np.float32)
